# revision 6
# baseline (speedup 1.0000x reference)
"""Single-head causal self-attention on 8 TRN2 NeuronCores.

Problem: embeddings [8, 4096, 1024], Wq/Wk/Wv [64, 1024] (fp32).
Sharding: data-parallel over batch — one batch element per core.

Per-core dataflow (T=4096, E=1024, A=64; fp32 data, float32r matmuls):
  Phase A (projection), per 512-row t-chunk:
    - DMA x rows naturally [128t, 1024e]; PE-transpose 128x128 blocks to
      build xT [128e, 8j, 512t] (fp32 has no DMA-transpose path).
    - psum_qk[128,512] = sum_j WqkT_j.T @ xT_j  -> rows 0:64 = q^T, 64:128 = k^T
    - psum_v [64,512]  = sum_j WvT_j.T  @ xT_j  -> v^T; PE-transpose back to
      v natural [128t, 64a] and append a ones column (v_aug [128, 65]).
  Phase B (attention), per 512-col q-chunk, streaming over k'-tiles j:
    - S^T tile = kT_j.T @ qT  (psum [128k', <=512q]); only causal columns.
    - E = exp(0.125 * S^T) on ACT; diagonal tiles masked by upper-tri x E.
    - out_aug^T [65, 512] += v_aug_j.T @ E   (ones column accumulates the
      softmax denominator, so no max-subtraction pass is needed; scores are
      ~N(0,1) so exp cannot overflow).
    - PE-transpose out_aug^T -> [128q, 65], divide by the denominator column,
      DMA out.
Phase A work for chunk c+1 is interleaved into phase B(c)'s k'-loop so the
tensor engine fills its exp-wait gaps and the activation engine never idles.
"""

import numpy as np

import concourse.bass as bass
import concourse.tile as tile
from concourse import bacc, mybir
from concourse.bass_utils import run_bass_kernel_spmd
from concourse.masks import make_identity, make_upper_triangular

B, T, E, A = 8, 4096, 1024, 64
NCORES = 8
TC = 512            # chunk size (t for phase A, q for phase B)
NCHUNK = T // TC    # 8
NJ = E // 128       # 8 e-slices
NT = T // 128       # 32 k'-tiles
FP = mybir.dt.float32
F32R = mybir.dt.float32r


def _build_attention(tc: tile.TileContext, out, x, wqk, wv):
    from contextlib import ExitStack

    nc = tc.nc
    with ExitStack() as ctx:
        const = ctx.enter_context(tc.tile_pool(name="const", bufs=1))
        identity = const.tile([128, 128], FP)
        make_identity(nc, identity)
        tri_f = const.tile([128, 128], FP)
        make_upper_triangular(nc, tri_f, val=1.0, diag=True)
        tri = const.tile([128, 128], F32R)
        nc.vector.tensor_copy(tri, tri_f)
        w_qk = const.tile([128, NJ, 128], F32R)
        nc.sync.dma_start(w_qk, wqk)
        w_v = const.tile([128, NJ, A], F32R)
        nc.sync.dma_start(w_v, wv)

        qT = const.tile([64, T], F32R)
        kT = const.tile([64, T], F32R)
        vsb = const.tile([128, NT, A + 1], F32R)
        ones = const.tile([128, 1], FP)
        nc.vector.memset(ones, 1.0)
        for jt in range(NT):
            nc.vector.tensor_copy(vsb[:, jt, A : A + 1], ones)

        xpool = ctx.enter_context(tc.tile_pool(name="xin", bufs=3))
        xTpool = ctx.enter_context(tc.tile_pool(name="xT", bufs=2))
        epool = ctx.enter_context(tc.tile_pool(name="ex", bufs=3))
        vtpool = ctx.enter_context(tc.tile_pool(name="vt", bufs=2))
        otpool = ctx.enter_context(tc.tile_pool(name="ot", bufs=2))
        opool = ctx.enter_context(tc.tile_pool(name="oseg", bufs=2))

        ps_tp = ctx.enter_context(tc.tile_pool(name="ps_tp", bufs=3, space="PSUM"))
        ps_mm = ctx.enter_context(tc.tile_pool(name="ps_mm", bufs=2, space="PSUM"))
        ps_s = ctx.enter_context(tc.tile_pool(name="ps_s", bufs=2, space="PSUM"))
        ps_o = ctx.enter_context(tc.tile_pool(name="ps_o", bufs=1, space="PSUM"))

        def phase_a_items(c):
            """Work-item closures for projections of chunk c (emit in order)."""
            items = []
            xT = xTpool.tile([128, NJ, TC], F32R, tag="xT")
            state = {}

            for tt in range(TC // 128):
                def dma_x(tt=tt):
                    x_t = xpool.tile([128, E], FP, tag="x")
                    state[tt] = x_t
                    r0 = c * TC + tt * 128
                    nc.sync.dma_start(x_t, x[r0 : r0 + 128, :])
                items.append(dma_x)
                for j in range(NJ):
                    def tp_x(tt=tt, j=j):
                        pxt = ps_tp.tile([128, 128], FP, tag="tp")
                        nc.tensor.transpose(
                            pxt, state[tt][:, j * 128 : (j + 1) * 128], identity
                        )
                        nc.vector.tensor_copy(
                            xT[:, j, tt * 128 : (tt + 1) * 128], pxt
                        )
                    items.append(tp_x)

            def mm_qk():
                pqk = ps_mm.tile([128, TC], FP, tag="mm")
                state["qk"] = pqk
                for j in range(NJ):
                    nc.tensor.matmul(
                        pqk, w_qk[:, j, :],
                        xT[:, j, :],
                        start=(j == 0), stop=(j == NJ - 1),
                    )
            items.append(mm_qk)

            def cp_qk():
                pqk = state["qk"]
                nc.vector.tensor_copy(qT[:, c * TC : (c + 1) * TC], pqk[0:64, :])
                nc.vector.tensor_copy(kT[:, c * TC : (c + 1) * TC], pqk[64:128, :])
            items.append(cp_qk)

            def mm_v():
                pv = ps_mm.tile([128, TC], FP, tag="mm")
                for j in range(NJ):
                    nc.tensor.matmul(
                        pv[0:64, :], w_v[:, j, :],
                        xT[:, j, :],
                        start=(j == 0), stop=(j == NJ - 1),
                    )
                vt_tmp = vtpool.tile([64, TC], FP, tag="vt")
                nc.vector.tensor_copy(vt_tmp, pv[0:64, :])
                state["vt"] = vt_tmp
            items.append(mm_v)

            for m in range(TC // 128):
                def tp_v(m=m):
                    pvt = ps_tp.tile([128, 128], FP, tag="tp")
                    nc.tensor.transpose(
                        pvt[:, 0:64],
                        state["vt"][:, m * 128 : (m + 1) * 128],
                        identity[0:64, 0:64],
                    )
                    nc.vector.tensor_copy(vsb[:, c * 4 + m, 0:A], pvt[:, 0:64])
                items.append(tp_v)
            return items

        def phase_b(c, fill_items):
            """Attention for q-chunk c; pops fill_items between iterations."""
            po = ps_o.tile([128, TC], FP, tag="o")
            njt = 4 * c + 4
            nfill = len(fill_items)
            done = 0
            for j in range(njt):
                d = max(0, j * 128 - c * TC)
                pss = ps_s.tile([128, TC], FP, tag="s")
                nc.tensor.matmul(
                    pss[:, d:],
                    kT[:, j * 128 : (j + 1) * 128],
                    qT[:, c * TC + d : (c + 1) * TC],
                    start=True, stop=True,
                )
                et = epool.tile([128, TC], F32R, tag="e")
                nc.scalar.activation(
                    et[:, d:], pss[:, d:],
                    mybir.ActivationFunctionType.Exp, scale=0.125,
                )
                if j >= 4 * c:
                    nc.vector.tensor_mul(
                        et[:, d : d + 128], et[:, d : d + 128], tri
                    )
                nc.tensor.matmul(
                    po[0 : A + 1, d:], vsb[:, j, :],
                    et[:, d:],
                    start=(j == 0), stop=(j == njt - 1),
                )
                # software-pipeline: emit next chunk's projection work here
                want = (j + 1) * nfill // njt
                while done < want:
                    fill_items[done]()
                    done += 1
            while done < nfill:
                fill_items[done]()
                done += 1

            ot_tmp = otpool.tile([A + 1, TC], FP, tag="otmp")
            nc.vector.tensor_copy(ot_tmp, po[0 : A + 1, :])
            oo = opool.tile([128, TC // 128, A], FP, tag="oo")
            for m in range(TC // 128):
                pot = ps_tp.tile([128, 128], FP, tag="tp")
                nc.tensor.transpose(
                    pot[:, 0 : A + 1],
                    ot_tmp[:, m * 128 : (m + 1) * 128],
                    identity[0 : A + 1, 0 : A + 1],
                )
                oseg = opool.tile([128, A + 1], FP, tag="os")
                nc.vector.tensor_copy(oseg, pot[:, 0 : A + 1])
                rec = opool.tile([128, 1], FP, tag="rec")
                nc.vector.reciprocal(rec, oseg[:, A : A + 1])
                nc.vector.tensor_scalar_mul(oo[:, m, :], oseg[:, 0:A], rec)
            nc.sync.dma_start(
                out[c * TC : (c + 1) * TC, :].rearrange(
                    "(m p) a -> p m a", p=128
                ),
                oo,
            )

        for it in phase_a_items(0):
            it()
        for c in range(NCHUNK):
            nxt = phase_a_items(c + 1) if c + 1 < NCHUNK else []
            phase_b(c, nxt)


_NC_CACHE = None


def _get_nc():
    global _NC_CACHE
    if _NC_CACHE is None:
        nc = bacc.Bacc(
            "TRN2",
            target_bir_lowering=False,
            debug=False,
            enable_asserts=True,
            num_devices=NCORES,
        )
        x = nc.dram_tensor("x", [T, E], FP, kind="ExternalInput").ap()
        wqk = nc.dram_tensor("wqk", [128, NJ, 128], F32R, kind="ExternalInput").ap()
        wv = nc.dram_tensor("wv", [128, NJ, A], F32R, kind="ExternalInput").ap()
        out = nc.dram_tensor("out", [T, A], FP, kind="ExternalOutput").ap()
        with tile.TileContext(nc) as tc:
            _build_attention(tc, out, x, wqk, wv)
        nc.compile()
        _NC_CACHE = nc
    return _NC_CACHE


def _make_in_maps(embeddings, Wq, Wk, Wv):
    # W_qkT[e, 0:64] = Wq[:, e].T, [64:128] = Wk -> sliced per 128-e block
    w_qk = np.concatenate([Wq, Wk], axis=0).T  # [E, 128]
    w_qk = np.ascontiguousarray(
        w_qk.reshape(NJ, 128, 128).transpose(1, 0, 2)
    )  # [128e_in_j, j, 128qk]
    w_v = np.ascontiguousarray(
        Wv.T.reshape(NJ, 128, A).transpose(1, 0, 2)
    )  # [128e_in_j, j, 64]
    return [
        {
            "x": np.ascontiguousarray(embeddings[i]),
            "wqk": w_qk,
            "wv": w_v,
        }
        for i in range(NCORES)
    ]


def run_on_hw(embeddings, Wq, Wk, Wv, trace=False):
    nc = _get_nc()
    in_maps = _make_in_maps(
        np.asarray(embeddings, dtype=np.float32),
        np.asarray(Wq, dtype=np.float32),
        np.asarray(Wk, dtype=np.float32),
        np.asarray(Wv, dtype=np.float32),
    )
    res = run_bass_kernel_spmd(nc, in_maps, list(range(NCORES)), trace=trace)
    out = np.stack([res.results[i]["out"] for i in range(NCORES)], axis=0)
    return out, res


def kernel(embeddings, Wq, Wk, Wv):
    out, _ = run_on_hw(embeddings, Wq, Wk, Wv, trace=False)
    return out


# ---------------------------------------------------------------------------
# Cached-jit runner: same execution path as run_bass_kernel_spmd's axon
# redirect (bass2jax.run_bass_via_pjrt), but the jitted shard_map callable is
# built once so warm calls skip retracing/recompiling. Used by test.py for
# steady-state timing; behavior identical to kernel().
# ---------------------------------------------------------------------------
_RUNNER = None


def _get_runner():
    global _RUNNER
    if _RUNNER is not None:
        return _RUNNER
    import jax
    from jax.sharding import Mesh, PartitionSpec
    from jax.experimental.shard_map import shard_map
    from concourse import bass2jax

    nc = _get_nc()
    bass2jax.install_neuronx_cc_hook()
    in_names, out_names, out_avals, zero_outs = [], [], [], []
    for alloc in nc.m.functions[0].allocations:
        if not isinstance(alloc, mybir.MemoryLocationSet):
            continue
        name = alloc.memorylocations[0].name
        if alloc.kind == "ExternalInput":
            in_names.append(name)
        elif alloc.kind == "ExternalOutput":
            shape = tuple(alloc.tensor_shape)
            dtype = mybir.dt.np(alloc.dtype)
            out_names.append(name)
            out_avals.append(jax.core.ShapedArray(shape, dtype))
            zero_outs.append(np.zeros(shape, dtype))
    n_params = len(in_names)
    n_outs = len(out_names)
    all_names = in_names + out_names
    donate = tuple(range(n_params, n_params + n_outs))

    def _body(*args):
        outs = bass2jax._bass_exec_p.bind(
            *args,
            out_avals=tuple(out_avals),
            in_names=tuple(all_names),
            out_names=tuple(out_names),
            lowering_input_output_aliases=(),
            sim_require_finite=True,
            sim_require_nnan=True,
            nc=nc,
        )
        return tuple(outs)

    devices = jax.devices()[:NCORES]
    mesh = Mesh(np.asarray(devices), ("core",))
    sharded = jax.jit(
        shard_map(
            _body,
            mesh=mesh,
            in_specs=(PartitionSpec("core"),) * (n_params + n_outs),
            out_specs=(PartitionSpec("core"),) * n_outs,
            check_rep=False,
        ),
        donate_argnums=donate,
        keep_unused=True,
    )

    def run(in_maps):
        concat_in = [
            np.concatenate([np.asarray(m[name]) for m in in_maps], axis=0)
            for name in in_names
        ]
        concat_zeros = [
            np.zeros((NCORES * z.shape[0], *z.shape[1:]), z.dtype)
            for z in zero_outs
        ]
        out_arrs = sharded(*concat_in, *concat_zeros)
        return [
            {
                name: np.asarray(out_arrs[i]).reshape(
                    NCORES, *out_avals[i].shape
                )[c]
                for i, name in enumerate(out_names)
            }
            for c in range(NCORES)
        ]

    _RUNNER = run
    return _RUNNER


def run_cached(embeddings, Wq, Wk, Wv):
    run = _get_runner()
    in_maps = _make_in_maps(
        np.asarray(embeddings, dtype=np.float32),
        np.asarray(Wq, dtype=np.float32),
        np.asarray(Wk, dtype=np.float32),
        np.asarray(Wv, dtype=np.float32),
    )
    results = run(in_maps)
    return np.stack([results[i]["out"] for i in range(NCORES)], axis=0)


# revision 16
# speedup vs baseline: 1.0828x; 1.0828x over previous
"""Single-head causal self-attention on 8 TRN2 NeuronCores.

Problem: embeddings [8, 4096, 1024], Wq/Wk/Wv [64, 1024] (fp32).
Sharding: data-parallel over batch — one batch element per core.

Per-core dataflow (T=4096, E=1024, A=64; fp32 data, float32r matmuls —
float32r is TRN2's full-rate fp32 matmul mode, ~11-bit mantissa rounding):
  Phase A (projection), per 512-row t-chunk:
    - DMA x rows naturally [128t, 1024e]; PE-transpose 128x128 blocks to
      build xT [128e, 8j, 512t] (fp32 has no DMA-transpose path).
    - psum_qk[128,512] = sum_j WqkT_j.T @ xT_j  -> rows 0:64 = q^T, 64:128 = k^T
    - psum_v [64,512]  = sum_j WvT_j.T  @ xT_j  -> v^T; PE-transpose back to
      v natural [128t, 64a] and append a ones column (v_aug [128, 65]).
  Phase B (attention), per 512-col q-chunk, streaming over k'-tiles j:
    - S^T tile = kT_j.T @ qT  (psum [128k', <=512q]); only causal columns.
    - E = exp(0.125 * S^T) on ACT; diagonal tiles masked by upper-tri x E.
    - out_aug^T [65, 512] += v_aug_j.T @ E   (ones column accumulates the
      softmax denominator, so no max-subtraction pass is needed; scores are
      ~N(0,1) so exp cannot overflow).
    - PE-transpose out_aug^T -> [128q, 65], divide by the denominator column,
      DMA out.
Phase A work for chunk c+1 is interleaved into phase B(c)'s k'-loop so the
tensor engine fills its exp-wait gaps and the activation engine never idles.
"""

import numpy as np

import concourse.bass as bass
import concourse.tile as tile
from concourse import bacc, mybir
from concourse.bass_utils import run_bass_kernel_spmd
from concourse.masks import make_identity, make_upper_triangular

B, T, E, A = 8, 4096, 1024, 64
NCORES = 8
TC = 512            # chunk size (t for phase A, q for phase B)
NCHUNK = T // TC    # 8
NJ = E // 128       # 8 e-slices
NT = T // 128       # 32 k'-tiles
FP = mybir.dt.float32
F32R = mybir.dt.float32r


def _build_attention(tc: tile.TileContext, out, x, wqk, wv):
    from contextlib import ExitStack

    nc = tc.nc
    with ExitStack() as ctx:
        const = ctx.enter_context(tc.tile_pool(name="const", bufs=1))
        identity = const.tile([128, 128], FP)
        make_identity(nc, identity)
        tri_f = const.tile([128, 128], FP)
        make_upper_triangular(nc, tri_f, val=1.0, diag=True)
        tri = const.tile([128, 128], F32R)
        nc.vector.tensor_copy(tri, tri_f)
        w_qk = const.tile([128, NJ, 128], F32R)
        nc.sync.dma_start(w_qk, wqk)
        w_v = const.tile([128, NJ, A], F32R)
        nc.sync.dma_start(w_v, wv)

        qT = const.tile([64, T], F32R)
        kT = const.tile([64, T], F32R)
        vsb = const.tile([128, NT, A + 1], F32R)
        ones = const.tile([128, 1], FP)
        nc.vector.memset(ones, 1.0)
        for jt in range(NT):
            nc.vector.tensor_copy(vsb[:, jt, A : A + 1], ones)

        xpool = ctx.enter_context(tc.tile_pool(name="xin", bufs=3))
        xTpool = ctx.enter_context(tc.tile_pool(name="xT", bufs=2))
        epool = ctx.enter_context(tc.tile_pool(name="ex", bufs=3))
        vtpool = ctx.enter_context(tc.tile_pool(name="vt", bufs=2))
        otpool = ctx.enter_context(tc.tile_pool(name="ot", bufs=2))
        opool = ctx.enter_context(tc.tile_pool(name="oseg", bufs=2))

        ps_tp = ctx.enter_context(tc.tile_pool(name="ps_tp", bufs=3, space="PSUM"))
        ps_mm = ctx.enter_context(tc.tile_pool(name="ps_mm", bufs=2, space="PSUM"))
        ps_s = ctx.enter_context(tc.tile_pool(name="ps_s", bufs=2, space="PSUM"))
        ps_o = ctx.enter_context(tc.tile_pool(name="ps_o", bufs=1, space="PSUM"))

        def phase_a_items(c):
            """Work-item closures for projections of chunk c (emit in order)."""
            items = []
            xT = xTpool.tile([128, NJ, TC], F32R, tag="xT", name="xT")
            state = {}

            for tt in range(TC // 128):
                def dma_x(tt=tt):
                    x_t = xpool.tile([128, E], FP, tag="x", name="x_t")
                    state[tt] = x_t
                    r0 = c * TC + tt * 128
                    nc.sync.dma_start(x_t, x[r0 : r0 + 128, :])
                items.append(dma_x)
                for j in range(NJ):
                    def tp_x(tt=tt, j=j):
                        pxt = ps_tp.tile([128, 128], FP, tag="tp", name="pxt")
                        nc.tensor.transpose(
                            pxt, state[tt][:, j * 128 : (j + 1) * 128], identity
                        )
                        nc.vector.tensor_copy(
                            xT[:, j, tt * 128 : (tt + 1) * 128], pxt
                        )
                    items.append(tp_x)

            def mm_qk():
                pqk = ps_mm.tile([128, TC], FP, tag="mm", name="pqk")
                state["qk"] = pqk
                for j in range(NJ):
                    nc.tensor.matmul(
                        pqk, w_qk[:, j, :], xT[:, j, :],
                        start=(j == 0), stop=(j == NJ - 1),
                    )
            items.append(mm_qk)

            def cp_qk():
                pqk = state["qk"]
                nc.vector.tensor_copy(qT[:, c * TC : (c + 1) * TC], pqk[0:64, :])
                nc.vector.tensor_copy(kT[:, c * TC : (c + 1) * TC], pqk[64:128, :])
            items.append(cp_qk)

            def mm_v():
                pv = ps_mm.tile([128, TC], FP, tag="mm", name="pv")
                for j in range(NJ):
                    nc.tensor.matmul(
                        pv[0:64, :], w_v[:, j, :], xT[:, j, :],
                        start=(j == 0), stop=(j == NJ - 1),
                    )
                vt_tmp = vtpool.tile([64, TC], FP, tag="vt", name="vt_tmp")
                nc.vector.tensor_copy(vt_tmp, pv[0:64, :])
                state["vt"] = vt_tmp
            items.append(mm_v)

            for m in range(TC // 128):
                def tp_v(m=m):
                    pvt = ps_tp.tile([128, 128], FP, tag="tp", name="pvt")
                    nc.tensor.transpose(
                        pvt[:, 0:64],
                        state["vt"][:, m * 128 : (m + 1) * 128],
                        identity[0:64, 0:64],
                    )
                    nc.vector.tensor_copy(vsb[:, c * 4 + m, 0:A], pvt[:, 0:64])
                items.append(tp_v)
            return items

        def phase_b(c, fill_items):
            """Attention for q-chunk c; pops fill_items between iterations."""
            po = ps_o.tile([128, TC], FP, tag="o", name="po")
            njt = 4 * c + 4
            nfill = len(fill_items)
            done = 0
            for j in range(njt):
                d = max(0, j * 128 - c * TC)
                pss = ps_s.tile([128, TC], FP, tag="s", name="pss")
                nc.tensor.matmul(
                    pss[:, d:],
                    kT[:, j * 128 : (j + 1) * 128],
                    qT[:, c * TC + d : (c + 1) * TC],
                    start=True, stop=True,
                )
                et = epool.tile([128, TC], F32R, tag="e", name="et")
                nc.scalar.activation(
                    et[:, d:], pss[:, d:],
                    mybir.ActivationFunctionType.Exp, scale=0.125,
                )
                if j >= 4 * c:
                    nc.vector.tensor_mul(
                        et[:, d : d + 128], et[:, d : d + 128], tri
                    )
                nc.tensor.matmul(
                    po[0 : A + 1, d:], vsb[:, j, :], et[:, d:],
                    start=(j == 0), stop=(j == njt - 1),
                )
                # software-pipeline: emit next chunk's projection work here
                want = (j + 1) * nfill // njt
                while done < want:
                    fill_items[done]()
                    done += 1
            while done < nfill:
                fill_items[done]()
                done += 1

            ot_tmp = otpool.tile([A + 1, TC], FP, tag="otmp", name="ot_tmp")
            nc.vector.tensor_copy(ot_tmp, po[0 : A + 1, :])
            oo = opool.tile([128, TC // 128, A], FP, tag="oo", name="oo")
            for m in range(TC // 128):
                pot = ps_tp.tile([128, 128], FP, tag="tp", name="pot")
                nc.tensor.transpose(
                    pot[:, 0 : A + 1],
                    ot_tmp[:, m * 128 : (m + 1) * 128],
                    identity[0 : A + 1, 0 : A + 1],
                )
                oseg = opool.tile([128, A + 1], FP, tag="os", name="oseg")
                nc.vector.tensor_copy(oseg, pot[:, 0 : A + 1])
                rec = opool.tile([128, 1], FP, tag="rec", name="rec")
                nc.vector.reciprocal(rec, oseg[:, A : A + 1])
                nc.vector.tensor_scalar_mul(oo[:, m, :], oseg[:, 0:A], rec)
            nc.sync.dma_start(
                out[c * TC : (c + 1) * TC, :].rearrange(
                    "(m p) a -> p m a", p=128
                ),
                oo,
            )

        for it in phase_a_items(0):
            it()
        for c in range(NCHUNK):
            nxt = phase_a_items(c + 1) if c + 1 < NCHUNK else []
            phase_b(c, nxt)


_NC_CACHE = None


def _get_nc():
    global _NC_CACHE
    if _NC_CACHE is None:
        nc = bacc.Bacc(
            "TRN2",
            target_bir_lowering=False,
            debug=False,
            enable_asserts=True,
            num_devices=NCORES,
        )
        x = nc.dram_tensor("x", [T, E], FP, kind="ExternalInput").ap()
        wqk = nc.dram_tensor("wqk", [128, NJ, 128], F32R, kind="ExternalInput").ap()
        wv = nc.dram_tensor("wv", [128, NJ, A], F32R, kind="ExternalInput").ap()
        out = nc.dram_tensor("out", [T, A], FP, kind="ExternalOutput").ap()
        with tile.TileContext(nc) as tc:
            _build_attention(tc, out, x, wqk, wv)
        nc.compile()
        _NC_CACHE = nc
    return _NC_CACHE


def _make_in_maps(embeddings, Wq, Wk, Wv):
    # W_qkT[e, 0:64] = Wq[:, e].T, [64:128] = Wk -> sliced per 128-e block
    w_qk = np.concatenate([Wq, Wk], axis=0).T  # [E, 128]
    w_qk = np.ascontiguousarray(
        w_qk.reshape(NJ, 128, 128).transpose(1, 0, 2)
    )  # [128e_in_j, j, 128qk]
    w_v = np.ascontiguousarray(
        Wv.T.reshape(NJ, 128, A).transpose(1, 0, 2)
    )  # [128e_in_j, j, 64]
    return [
        {
            "x": np.ascontiguousarray(embeddings[i]),
            "wqk": w_qk,
            "wv": w_v,
        }
        for i in range(NCORES)
    ]


def run_on_hw(embeddings, Wq, Wk, Wv, trace=False):
    nc = _get_nc()
    in_maps = _make_in_maps(
        np.asarray(embeddings, dtype=np.float32),
        np.asarray(Wq, dtype=np.float32),
        np.asarray(Wk, dtype=np.float32),
        np.asarray(Wv, dtype=np.float32),
    )
    res = run_bass_kernel_spmd(nc, in_maps, list(range(NCORES)), trace=trace)
    out = np.stack([res.results[i]["out"] for i in range(NCORES)], axis=0)
    return out, res


def kernel(embeddings, Wq, Wk, Wv):
    out, _ = run_on_hw(embeddings, Wq, Wk, Wv, trace=False)
    return out
